# revision 17
# baseline (speedup 1.0000x reference)
"""AnalyticHashLinear Trainium2 kernel (8 NeuronCores, batch-sharded SPMD).

y = x @ W^T + bias,  W[o,i] = cb[(c0 + 10007*o + 20011*i) mod 2^16] * (-1)^(o+i+1)

Key algebra (all mod 2^16):
  inv(20011) = 131, 131*10007 = 197, c0 = 3*40009 mod 2^16 = 54491, t0 = 131*c0 = 60433
  P1[j] := cb[20011*j]  =>  W[o,i] = P1[t0 + 197*o + i]
  signs: (-1)^(o+i) = (-1)^(j - t0)  (197 odd) -> folded into table P1s.
  Table build uses the run structure P1[j0 + 131*k] = cb[20011*j0 + k] (contiguous
  codebook runs), so the permutation is done with a few chunky DMAs + 8 xbar
  transposes instead of 65536 scattered elements.
  W^T tiles [i-part, o-free] = xbar-transposed windows  in[p_o, f_i] =
  P1s_rep[base + 197*p_o + f_i].

Each core: 1024 rows of x, full out_dim. No collectives.
"""
import os
import numpy as np
import ml_dtypes

import concourse.bass as bass
import concourse.tile as tile
import concourse.mybir as mybir

F32 = mybir.dt.float32
BF16 = mybir.dt.bfloat16
I16 = mybir.dt.int16
nbf16 = ml_dtypes.bfloat16

MOD = 65536
HA, HB, HC = 10007, 20011, 40009
LAYER = 3
C0 = (LAYER * HC) % MOD            # 54491
U = pow(HB, -1, MOD)               # 131
S = (U * HA) % MOD                 # 197
T0 = (U * C0) % MOD                # 60433

NB = 1024                          # batch rows per core
IN_DIM = 4096
OUT_DIM = 4096
KT = IN_DIM // 128                 # 32 contraction tiles
TBL = 95232                        # P1s_rep length (>= 65535 + 127*197 + 4095 + 128)
CBREP = 2625536                    # cb16 replicated length (covers 130*20011 + 20011*16 view)
OC = 1024                          # o-chunk
NOC = OUT_DIM // OC                # 4
WT_BUFS = 34


def _consts():
    # eps on T1 [j0-part, k-free]: +1 iff (j0 + k) odd
    p = np.arange(128)[:, None]
    f = np.arange(512)[None, :]
    eps1 = np.where(((p + f) % 2) == 1, 1.0, -1.0).astype(nbf16)
    # eps on T2 (j0 = 115 + p): +1 iff (p + f) even
    p2 = np.arange(16)[:, None]
    eps2 = np.where(((p2 + f) % 2) == 0, 1.0, -1.0).astype(nbf16)
    return eps1, eps2


def build_kernel(tc: tile.TileContext, out_h, x_h, cb_h, bias_h):
    nc = tc.nc
    eps1_np, eps2_np = _consts()
    eps1_h = nc.inline_tensor(eps1_np, name="eps1")
    eps2_h = nc.inline_tensor(eps2_np, name="eps2")

    cbrep_h = nc.dram_tensor("cbrep", [CBREP], BF16, kind="Internal")
    p1_h = nc.dram_tensor("p1rep", [TBL], BF16, kind="Internal")
    xb_h = nc.dram_tensor("xb16", [NB, IN_DIM], BF16, kind="Internal")
    brep_h = nc.dram_tensor("brep", [128 * OUT_DIM], F32, kind="Internal")

    from contextlib import ExitStack
    with ExitStack() as ctx:
        build_pool = ctx.enter_context(tc.tile_pool(name="build", bufs=1))
        const_pool = ctx.enter_context(tc.tile_pool(name="const", bufs=1))
        xt_pool = ctx.enter_context(tc.tile_pool(name="xt", bufs=1))
        wt_pool = ctx.enter_context(tc.tile_pool(name="wt", bufs=WT_BUFS))
        y_pool = ctx.enter_context(tc.tile_pool(name="y", bufs=3))
        ps_pool = ctx.enter_context(tc.tile_pool(name="ps", bufs=4, space="PSUM"))

        # ---- gpsimd cast DMAs (f32 -> bf16) ----
        nc.gpsimd.dma_start(cbrep_h[0:MOD], cb_h[:])
        nc.gpsimd.dma_start(xb_h[:, :].flatten(), x_h[:, :].flatten())

        # ---- x^T via xbar transposes: xT[:, kt*NB + b] = xb16[b, kt*128 + p]
        xT = xt_pool.tile([128, KT * NB], BF16)
        heng = [nc.sync, nc.scalar]
        for kt in range(KT):
            for bt in range(NB // 128):
                heng[(kt * (NB // 128) + bt) % 2].dma_start(
                    xT[:, kt * NB + bt * 128: kt * NB + (bt + 1) * 128],
                    xb_h[bt * 128:(bt + 1) * 128, kt * 128:(kt + 1) * 128],
                    transpose=True,
                )

        # ---- bias broadcast [128, 4096] f32 via DRAM doubling ----
        nc.sync.dma_start(brep_h[0:OUT_DIM], bias_h[:])
        nrep = OUT_DIM
        while nrep < 128 * OUT_DIM:
            nc.sync.dma_start(brep_h[nrep:2 * nrep], brep_h[0:nrep])
            nrep *= 2
        bias_bc = const_pool.tile([128, OUT_DIM], F32)
        nc.sync.dma_start(
            bias_bc[:], brep_h[:].rearrange("(p f) -> p f", f=OUT_DIM)
        )

        # ---- table build ----
        n = MOD
        while n < CBREP:
            m = min(n, CBREP - n)
            nc.sync.dma_start(cbrep_h[n:n + m], cbrep_h[0:m])
            n += m

        T1 = build_pool.tile([128, 512], BF16, tag="T1")
        nc.sync.dma_start(
            T1[:],
            cbrep_h[0:128 * HB].rearrange("(p s) -> p s", s=HB)[:, 0:512],
        )
        T2 = build_pool.tile([16, 512], BF16, tag="T2")
        b2 = 115 * HB
        nc.sync.dma_start(
            T2[:],
            cbrep_h[b2:b2 + 16 * HB].rearrange("(p s) -> p s", s=HB)[:, 0:512],
        )
        eps1_sb = build_pool.tile([128, 512], BF16, tag="e1")
        nc.sync.dma_start(eps1_sb[:], eps1_h[:, :])
        eps2_sb = build_pool.tile([16, 512], BF16, tag="e2")
        nc.sync.dma_start(eps2_sb[:], eps2_h[:, :])
        V1 = build_pool.tile([128, 512], BF16, tag="V1")
        nc.vector.tensor_mul(V1[:], T1[:], eps1_sb[:])
        V2 = build_pool.tile([16, 512], BF16, tag="V2")
        nc.vector.tensor_mul(V2[:], T2[:], eps2_sb[:])

        for c in range(4):
            U1 = build_pool.tile([128, 128], BF16, tag="U1")
            nc.sync.dma_start(U1[:], V1[:, 128 * c:128 * (c + 1)], transpose=True)
            off = 131 * 128 * c
            nc.sync.dma_start(
                p1_h[off:off + 131 * 128].rearrange("(p s) -> p s", s=131)[:, 0:128],
                U1[:],
            )
            U2 = build_pool.tile([128, 16], BF16, tag="U2")
            nc.sync.dma_start(U2[:], V2[:, 128 * c:128 * (c + 1)], transpose=True)
            off2 = 115 + 131 * 128 * c
            nc.sync.dma_start(
                p1_h[off2:off2 + 131 * 128].rearrange("(p s) -> p s", s=131)[:, 0:16],
                U2[:],
            )
        # tail replicate (positions >= 65536 mirror j - 65536)
        nc.sync.dma_start(p1_h[MOD:TBL], p1_h[0:TBL - MOD])

        # ---- main loop ----
        for oc in range(NOC):
            wt = []
            for kt in range(KT):
                w = wt_pool.tile([128, OC], BF16, tag="wt")
                for j in range(OC // 128):
                    o0 = oc * OC + j * 128
                    base = ((T0 + S * o0) % MOD) + kt * 128
                    win = p1_h[base:base + S * 128].rearrange(
                        "(p s) -> p s", s=S
                    )[:, 0:128]
                    heng[(kt * (OC // 128) + j) % 2].dma_start(
                        w[:, 128 * j:128 * (j + 1)], win, transpose=True)
                wt.append(w)
            for bt in range(NB // 128):
                ps0 = ps_pool.tile([128, 512], F32, tag="ps")
                ps1 = ps_pool.tile([128, 512], F32, tag="ps")
                for kt in range(KT):
                    xoff = kt * NB + bt * 128
                    lhsT = xT[:, xoff:xoff + 128]
                    nc.tensor.matmul(
                        ps0[:], lhsT, wt[kt][:, 0:512],
                        start=(kt == 0), stop=(kt == KT - 1),
                    )
                    nc.tensor.matmul(
                        ps1[:], lhsT, wt[kt][:, 512:OC],
                        start=(kt == 0), stop=(kt == KT - 1),
                    )
                for h, ps in ((0, ps0), (1, ps1)):
                    ob = oc * OC + h * 512
                    yt = y_pool.tile([128, 512], F32, tag="y")
                    nc.vector.tensor_add(yt[:], ps[:], bias_bc[:, ob:ob + 512])
                    nc.sync.dma_start(
                        out_h[bt * 128:(bt + 1) * 128, ob:ob + 512], yt[:]
                    )


# This container's walrus rejects the EVENT_SEMAPHORE_RANGE_CLEAR ISA encoding
# ("ISA wrong length") that TileContext emits when freeing semaphores at kernel
# exit. The preamble zeroes all semaphore banks via InstMemset at the start of
# every execution, so the exit-time clear is redundant — skip emitting it but
# keep the allocator bookkeeping.
def _patched_clear_and_free_semaphores(self, sems):
    if not sems:
        return
    sem_nums = [
        sem.num if isinstance(sem, bass.SemaphoreHandle) else sem for sem in sems
    ]
    self._state.prepend_free_semaphores(sem_nums)
    for poison_set in self._tile_sem_poison_stack:
        poison_set.update(sem_nums)


bass.Bass.clear_and_free_semaphores = _patched_clear_and_free_semaphores


# Same walrus also only encodes ONE sync-wait on non-EventSemaphore
# instructions ("Too many sync wait commands"), but TileContext's kernel-tail
# drain gets one wait per outstanding DMA sem lane. Split the extras onto
# additional drains (sequential on the same engine => same semantics).
def _patched_drain_and_barrier(self, tick_clock, wait_clock):
    import bass_rust as _br
    from concourse.vector_clock import ScopedClock

    nc = self.nc
    drain_inst = nc.sync.drain()
    wait_clock.add_sem_waits(
        drain_inst.ins, ScopedClock({None: tick_clock.global_clock})
    )
    si = drain_inst.ins.sync_info
    if si is not None and si.on_wait and len(si.on_wait) > 1:
        waits = list(si.on_wait)
        si.on_wait = waits[:1]
        for w in waits[1:]:
            d2 = nc.sync.drain()
            d2.ins.sync_info = _br.SyncInfo(on_wait=[w], on_update=[])
    nc.all_engine_barrier()
    assert self.sems is not None
    popped = nc._tile_sem_poison_stack.pop()
    assert popped is self._sem_poison
    nc.clear_and_free_semaphores(list(self.sems.allocated().values()))
    nc.all_engine_barrier()


tile.TileContext._drain_and_barrier = _patched_drain_and_barrier


def _split_multiwait(nc):
    """Walrus in this container encodes at most 1 sync-wait per instruction
    (2 for EventSemaphore). Tile's scheduler attaches more. Move extra waits
    onto InstNoOp carriers inserted just before the instruction in its block
    (same engine => executes in order => identical semantics)."""
    import bass_rust as _br

    for f in nc.m.functions:
        for blk in f.blocks:
            insts = blk.instructions
            i = 0
            while i < len(insts):
                inst = insts[i]
                si = getattr(inst, "sync_info", None)
                cap = 2 if type(inst).__name__ == "InstEventSemaphore" else 1
                if si is not None and si.on_wait and len(si.on_wait) > cap:
                    waits = list(si.on_wait)
                    si.on_wait = waits[:cap]
                    for w in waits[cap:]:
                        nop = nc.engines[inst.engine].nop()
                        nopi = nop.ins
                        nopi.sync_info = _br.SyncInfo(on_wait=[w], on_update=[])
                        # nop() appended itself to the current (last) block;
                        # move it to just before `inst`.
                        src_list = nc.cur_bb.bb.instructions
                        assert src_list[len(src_list) - 1].name == nopi.name
                        src_list.pop()
                        insts.insert(i, nopi)
                        i += 1
                i += 1

_NC_CACHE = None


def _build_nc():
    global _NC_CACHE
    if _NC_CACHE is not None:
        return _NC_CACHE
    nc = bass.Bass(trn_type="TRN2")
    x_h = nc.dram_tensor("x", [NB, IN_DIM], F32, kind="ExternalInput")
    cb_h = nc.dram_tensor("codebook", [MOD], F32, kind="ExternalInput")
    bias_h = nc.dram_tensor("bias", [OUT_DIM], F32, kind="ExternalInput")
    out_h = nc.dram_tensor("out", [NB, OUT_DIM], F32, kind="ExternalOutput")
    with tile.TileContext(nc) as tc:
        build_kernel(tc, out_h, x_h, cb_h, bias_h)
    _split_multiwait(nc)
    _NC_CACHE = nc
    return nc


def kernel(x, codebook, bias):
    x = np.ascontiguousarray(np.asarray(x, dtype=np.float32))
    codebook = np.ascontiguousarray(np.asarray(codebook, dtype=np.float32))
    bias = np.ascontiguousarray(np.asarray(bias, dtype=np.float32))
    assert x.shape == (8192, 4096)

    from concourse.bass_utils import run_bass_kernel_spmd

    nc = _build_nc()
    in_maps = [
        {"x": x[c * NB:(c + 1) * NB], "codebook": codebook, "bias": bias}
        for c in range(8)
    ]
    trace = os.environ.get("KERNEL_TRACE", "0") == "1"
    res = run_bass_kernel_spmd(nc, in_maps, core_ids=list(range(8)), trace=trace)
    if trace and res.exec_time_ns is not None:
        print(f"HW exec time: {res.exec_time_ns} ns")
    out = np.concatenate([r["out"] for r in res.results], axis=0)
    return out
